# revision 51
# baseline (speedup 1.0000x reference)
"""Trainium2 Bass kernel for masked sigmoid context attention.

Model (per batch b, with n = R*C = 4096 tokens, D = 512, H = 8 heads of d = 64):
    qh/kh/vh = heads(x @ W + b)
    attn = sigmoid(qh @ kh^T / 8) * mask_keys
    attn = attn / (eps + sum(mask))          # per-batch scalar
    out  = (attn @ vh heads-merged) @ Wo + bo + q

Sharding: 8 cores = 2 batches x 4 head-groups (2 heads / group).
Each core computes its group's projections, flash-style sigmoid attention
(sigmoid is elementwise -> no softmax bookkeeping), and a partial output
projection x_g @ Wo_g.  The host sums the 4 partials per batch and adds
bias + residual (pure unsharding; all matmul FLOPs run on device).

Key device-side choices (see engine docs):
  * masked keys are compacted away on the host (mask is ~50% zeros), and
    mask/denominator are folded into V rows, so masking costs nothing
  * all matmuls in bf16 with fp32 PSUM accumulation (residual dominates the
    output magnitude, so attention-path bf16 error is ~1e-4 relative)
  * q/k/v ship host-transposed (contraction dim on rows) so projections
    consume them directly -- zero on-chip transposes
  * QK^T (K=64) runs 2 heads concurrently via PE row-packing; attn@V (M=64)
    runs 2 heads concurrently via PE col-packing (measured concurrent on HW)
  * sigmoid on ScalarE from 2-bank PSUM tiles (FD=1024) to amortize overhead;
    ScalarE is the bottleneck engine (~142us/core) and runs ~97% utilized
"""

import math
import os
from contextlib import ExitStack

import ml_dtypes
import numpy as np

import concourse.bass as bass
import concourse.mybir as mybir
import concourse.tile as tile
from concourse import bacc
from concourse.bass import ts
from concourse.bass_utils import run_bass_kernel_spmd

F32 = mybir.dt.float32
BF16 = mybir.dt.bfloat16
BF = ml_dtypes.bfloat16

H = 8
DH = 64
D = 512
GD = 128           # head-group dim = 2 heads x 64
NQ = 4096          # tokens per batch
TEMP = 8.0
EPS = 1e-6
QB = 512           # query block for attention
N_CORES = 8

LAST_RESULT = None  # BassKernelResults of the most recent run (for test harness)
_NC_CACHE = {}


def _build_nc(KT: int, loop_n: int | None = None) -> bass.Bass:
    """Bass program for one core: batch slice + one head-group. KT = key tiles.

    loop_n: benchmarking aid -- wrap the whole kernel body in a hardware
    For_i loop so one NEFF execution runs the kernel loop_n times (used to
    measure per-iteration HW time through the remote-dispatch jitter).
    Not used for the normal kernel() path."""
    KM = KT * 128
    nc = bacc.Bacc(None)

    # q/k/v arrive HOST-TRANSPOSED (contraction dim D on rows) so the
    # projections can consume them directly -- no on-chip transposes
    xq = nc.declare_dram_parameter("xq", [D, NQ], BF16, isOutput=False)
    xk = nc.declare_dram_parameter("xk", [D, KM], BF16, isOutput=False)
    xv = nc.declare_dram_parameter("xv", [D, KM], BF16, isOutput=False)
    wq = nc.declare_dram_parameter("wq", [D, GD], BF16, isOutput=False)
    wk = nc.declare_dram_parameter("wk", [D, GD], BF16, isOutput=False)
    wv = nc.declare_dram_parameter("wv", [D, GD], BF16, isOutput=False)
    wo = nc.declare_dram_parameter("wo", [GD, D], BF16, isOutput=False)
    bq = nc.declare_dram_parameter("bq", [GD, 1], F32, isOutput=False)
    bk = nc.declare_dram_parameter("bk", [GD, 1], F32, isOutput=False)
    bv = nc.declare_dram_parameter("bv", [1, GD], BF16, isOutput=False)
    # per-key scale = mask/(eps+sum(mask))
    vs_p = nc.declare_dram_parameter("vs_p", [KM, 1], F32, isOutput=False)   # key on partition
    out = nc.declare_dram_parameter("out", [NQ, D], F32, isOutput=True)

    with tile.TileContext(nc) as tc, ExitStack() as ctx:
        if loop_n is not None:
            ctx.enter_context(tc.For_i(0, loop_n, 1))
        const = ctx.enter_context(tc.tile_pool(name="const", bufs=1))
        persist = ctx.enter_context(tc.tile_pool(name="persist", bufs=1))
        p_pool = ctx.enter_context(tc.tile_pool(name="p", bufs=6))
        out_pool = ctx.enter_context(tc.tile_pool(name="outs", bufs=4))
        psum_s = ctx.enter_context(tc.tile_pool(name="ps", bufs=2, space="PSUM"))
        psum_x = ctx.enter_context(tc.tile_pool(name="px", bufs=2, space="PSUM"))
        psum_misc = ctx.enter_context(tc.tile_pool(name="pm", bufs=2, space="PSUM"))

        # ---- inputs: transposed q/k/v as (128, 4chunk, n) sbuf tiles ----
        # k/v first halves lead the SP queue (they gate the first sigmoids);
        # big tails follow; v rides the SWDGE queue in parallel
        def load_tails():
            # big streaming tails go AFTER the weight/const loads so they
            # don't block the first projections in queue order
            if KM > KH:
                KH2 = min(2 * KH, KM)
                nc.sync.dma_start(xk_s[:, :, KH:KH2], xkr[:, :, KH:KH2])
                nc.gpsimd.dma_start(xv_s[:, :, KH:KH2], xvr[:, :, KH:KH2])
                if KM > KH2:
                    nc.sync.dma_start(xk_s[:, :, KH2:KM], xkr[:, :, KH2:KM])
                    nc.gpsimd.dma_start(xv_s[:, :, KH2:KM], xvr[:, :, KH2:KM])
            for h in range(1, 4):
                nc.sync.dma_start(xq_s[:, :, ts(h, 2 * QB)], xqr[:, :, ts(h, 2 * QB)])

        # ---- constants (weights ship pre-cast bf16) ---------------------
        def load_w_chunks(dram, name):  # (D, GD) -> sbuf (128, 4, GD) bf16
            b = const.tile([128, 4, GD], BF16, tag=name)
            nc.sync.dma_start(b[:], dram.rearrange("(c p) m -> p c m", p=128))
            return b

        KH = min(4, KT) * 128
        xk_s = persist.tile([128, 4, KM], BF16)
        xv_s = persist.tile([128, 4, KM], BF16)
        xq_s = persist.tile([128, 4, NQ], BF16)
        xkr = xk.rearrange("(c p) n -> p c n", p=128)
        xvr = xv.rearrange("(c p) n -> p c n", p=128)
        xqr = xq.rearrange("(c p) n -> p c n", p=128)
        nc.sync.dma_start(xk_s[:, :, 0:KH], xkr[:, :, 0:KH])
        nc.gpsimd.dma_start(xv_s[:, :, 0:KH], xvr[:, :, 0:KH])
        nc.sync.dma_start(xq_s[:, :, 0:2 * QB], xqr[:, :, 0:2 * QB])

        wq_b = load_w_chunks(wq, "wq_b")
        wk_b = load_w_chunks(wk, "wk_b")
        wv_b = load_w_chunks(wv, "wv_b")
        wo_b = const.tile([GD, D], BF16)
        nc.sync.dma_start(wo_b[:], wo[:, :])

        bq_s = const.tile([GD, 1], F32)
        nc.sync.dma_start(bq_s[:], bq[:, :])
        bk_s = const.tile([GD, 1], F32)
        nc.sync.dma_start(bk_s[:], bk[:, :])
        bv_b = const.tile([1, GD], BF16)
        nc.sync.dma_start(bv_b[:], bv[:, :])
        ones1 = const.tile([1, 128], BF16)
        nc.gpsimd.memset(ones1[:], 1.0)

        vsp_s = const.tile([128, KT], F32)
        nc.sync.dma_start(vsp_s[:], vs_p.rearrange("(t p) o -> p (t o)", p=128))
        load_tails()

        qhT = persist.tile([128, NQ], BF16)   # [h1 d | h2 d] on partitions
        khT = persist.tile([128, KM], BF16)
        vhB = persist.tile([128, KM], BF16)   # per ktile block: (key, group-col)
        xT = persist.tile([128, NQ], BF16)    # attention out, d on partitions

        # ---- projections (the host-transposed inputs feed PE directly) -
        def q_proj(qb):
            qsl = slice(qb * QB, (qb + 1) * QB)
            pp = psum_misc.tile([128, 512], F32, tag="pm_p")
            for c in range(4):
                nc.tensor.matmul(pp[:], lhsT=wq_b[:, c, :], rhs=xq_s[:, c, qsl],
                                 start=(c == 0), stop=(c == 3))
            nc.vector.tensor_scalar_add(qhT[:, qsl], pp[:], bq_s[:])

        def k_proj(g0, gs):
            ksl = slice(g0 * 128, (g0 + gs) * 128)
            pp = psum_misc.tile([128, 512], F32, tag="pm_p")
            for c in range(4):
                nc.tensor.matmul(pp[:, : gs * 128], lhsT=wk_b[:, c, :],
                                 rhs=xk_s[:, c, ksl], start=(c == 0), stop=(c == 3))
            nc.vector.tensor_scalar_add(khT[:, ksl], pp[:, : gs * 128], bk_s[:])

        def v_proj(g0, gs):
            # vhB[key, :] = ((v @ Wv_g) + bv) * vscale[key]; the row scale
            # commutes with the right-multiplication, so it rides the evac
            pv = psum_misc.tile([128, 512], F32, tag="pm_p")
            for j in range(gs):
                t = g0 + j
                for c in range(4):
                    nc.tensor.matmul(
                        pv[:, ts(j, 128)], lhsT=xv_s[:, c, ts(t, 128)],
                        rhs=wv_b[:, c, :], start=(c == 0), stop=False)
                # += 1 * bv  (rank-1 via K=1 matmul)
                nc.tensor.matmul(pv[:, ts(j, 128)], lhsT=ones1[:],
                                 rhs=bv_b[:], start=False, stop=True)
            for j in range(gs):
                t = g0 + j
                nc.vector.tensor_scalar_mul(
                    vhB[:, ts(t, 128)], pv[:, ts(j, 128)], vsp_s[:, t:t + 1])

        groups = [(g0, min(4, KT - g0)) for g0 in range(0, KT, 4)]
        q_proj(0)
        q_proj(1)
        for g0, gs in groups:
            k_proj(g0, gs)
            v_proj(g0, gs)

        # ---- attention + pipelined q-proj + output projection ----------
        def out_proj(qb):
            # partial output projection; stores ride the SWDGE queue so
            # they never head-of-line-block the SP load queue.  The last
            # block's stores are on the critical tail and the SP queue is
            # drained by then, so they go HWDGE instead.
            last = qb == NQ // QB - 1
            for j in range(4):
                nt = qb * 4 + j
                po = psum_misc.tile([128, 512], F32, tag="pm_p")
                nc.tensor.matmul(po[:], lhsT=xT[:, ts(nt, 128)], rhs=wo_b[:],
                                 start=True, stop=True)
                ot = out_pool.tile([128, D], F32, tag="ot")
                nc.vector.tensor_copy(ot[:], po[:])
                (nc.sync if last else nc.gpsimd).dma_start(
                    out[ts(nt, 128), :], ot[:])

        # outproj(qb) and q_proj(qb+2) are emitted a few tiles INTO block
        # qb+1 so they don't outrank qb+1's first s-matmul fills in the
        # scheduler's priority order (= emission order)
        deferred = None
        for qb in range(NQ // QB):
            qsl = slice(qb * QB, (qb + 1) * QB)
            xa = psum_x.tile([128, QB], F32, tag="px_x")  # h1 -> parts 0:64
            xb = psum_x.tile([128, QB], F32, tag="px_x")  # h2 -> parts 64:128
            for t in range(KT):
                sg = psum_s.tile([128, 1024], F32, tag="ps_t")
                # two K=64 matmuls in distinct PE row-groups, concurrent
                nc.tensor.matmul(sg[:, 0:512], lhsT=khT[0:64, ts(t, 128)],
                                 rhs=qhT[0:64, qsl], start=True, stop=True)
                nc.tensor.matmul(sg[:, 512:1024], lhsT=khT[64:128, ts(t, 128)],
                                 rhs=qhT[64:128, qsl], start=True, stop=True)
                p = p_pool.tile([128, 1024], BF16, tag="p")
                nc.scalar.activation(
                    p[:], sg[:], mybir.ActivationFunctionType.Sigmoid,
                    scale=1.0 / TEMP)
                # two M=64 matmuls in distinct PE col-groups, concurrent
                nc.tensor.matmul(
                    xa[0:64, :], lhsT=vhB[:, t * 128:t * 128 + 64],
                    rhs=p[:, 0:512], start=(t == 0), stop=(t == KT - 1))
                nc.tensor.matmul(
                    xb[64:128, :], lhsT=vhB[:, t * 128 + 64:t * 128 + 128],
                    rhs=p[:, 512:1024], start=(t == 0), stop=(t == KT - 1))
                if t == 3 and deferred is not None:
                    deferred()
                    deferred = None
            nc.vector.tensor_copy(xT[0:64, qsl], xa[0:64, :])
            nc.vector.tensor_copy(xT[64:128, qsl], xb[64:128, :])

            def make_deferred(qb=qb):
                def fn():
                    out_proj(qb)
                    if qb + 2 < NQ // QB:
                        q_proj(qb + 2)
                return fn
            deferred = make_deferred()
        deferred()

    nc.compile()
    return nc


def kernel(q, k, v, mask, Wq, bq, Wk, bk, Wv, bv, Wo, bo):
    global LAST_RESULT
    q = np.asarray(q, np.float32)
    k = np.asarray(k, np.float32)
    v = np.asarray(v, np.float32)
    mask = np.asarray(mask)
    B, R, C, D_ = q.shape
    n = R * C
    assert (n, D_) == (NQ, D)
    qf = q.reshape(B, n, D)
    kf = k.reshape(B, n, D)
    vf = v.reshape(B, n, D)
    mf = mask.reshape(B, n)
    counts = mf.sum(axis=1)
    KT = max(1, math.ceil(counts.max() / 128))
    KM = KT * 128

    if KT not in _NC_CACHE:
        _NC_CACHE[KT] = _build_nc(KT)
    nc = _NC_CACHE[KT]

    in_maps = []
    kc_b, vc_b, vsp_b, xq_b = [], [], [], []
    for b in range(B):
        idx = np.nonzero(mf[b])[0]
        nk = len(idx)
        kc = np.zeros((KM, D), np.float32)
        vc = np.zeros((KM, D), np.float32)
        kc[:nk] = kf[b, idx]
        vc[:nk] = vf[b, idx]
        vs = np.zeros((KM, 1), np.float32)
        vs[:nk] = 1.0 / (EPS + float(counts[b]))
        # ship transposed (contraction dim on rows) so the device consumes
        # them directly as matmul operands -- no on-chip transposes
        kc_b.append(np.ascontiguousarray(kc.astype(BF).T))
        vc_b.append(np.ascontiguousarray(vc.astype(BF).T))
        vsp_b.append(vs)
        xq_b.append(np.ascontiguousarray(qf[b].astype(BF).T))

    Wq = np.asarray(Wq, np.float32)
    Wk = np.asarray(Wk, np.float32)
    Wv = np.asarray(Wv, np.float32)
    Wo = np.asarray(Wo, np.float32)
    bqv = np.asarray(bq, np.float32)
    bkv = np.asarray(bk, np.float32)
    bvv = np.asarray(bv, np.float32)

    for core in range(N_CORES):
        b, g = divmod(core, N_CORES // B)
        gsl = slice(g * GD, (g + 1) * GD)
        in_maps.append(dict(
            xq=xq_b[b], xk=kc_b[b], xv=vc_b[b],
            wq=np.ascontiguousarray(Wq[:, gsl].astype(BF)),
            wk=np.ascontiguousarray(Wk[:, gsl].astype(BF)),
            wv=np.ascontiguousarray(Wv[:, gsl].astype(BF)),
            wo=np.ascontiguousarray(Wo[gsl, :].astype(BF)),
            bq=np.ascontiguousarray(bqv[gsl].reshape(GD, 1)),
            bk=np.ascontiguousarray(bkv[gsl].reshape(GD, 1)),
            bv=np.ascontiguousarray(bvv[gsl].reshape(1, GD).astype(BF)),
            vs_p=vsp_b[b],
        ))

    global _last_in_maps
    _last_in_maps = in_maps
    LAST_RESULT = run_bass_kernel_spmd(nc, in_maps, list(range(N_CORES)))
    results = LAST_RESULT.results

    bo = np.asarray(bo, np.float32)
    full = np.empty((B, n, D), np.float32)
    for b in range(B):
        acc = results[b * 4 + 0]["out"].astype(np.float32).copy()
        for g in range(1, 4):
            acc += results[b * 4 + g]["out"]
        full[b] = acc + bo[None, :] + qf[b]
    return full.reshape(B, R, C, D).astype(np.float32)
